# revision 52
# baseline (speedup 1.0000x reference)
"""Trainium2 Bass kernel for ragged GQA attention decode (B=16, QL=4, KV=4096,
H=32, KVH=8, D=128, DIM=4096), tensor-parallel over 8 NeuronCores.

Sharding: core c owns q-heads [4c, 4c+4) and kv-head c. wq/wk/wv are
column-split, wo row-split, KV cache split along the kv-head dim. Each core
computes a partial [64, 4096] output (its heads through its wo rows); the
host sums the 8 partials.

The Bass graph is specialized to the actual cache_len values (known on host
at build time), so only the live prefix of the KV cache is ever read.

Compute runs in bf16 (f32 PSUM accumulation): the weights and KV cache are
shipped to the device as bf16 shards, halving HBM traffic and making the
TensorEngine matmuls single-pass.
"""

import math
import sys
import types

import numpy as np

B, QL, KV, H, KVH, D, DIM = 16, 4, 4096, 32, 8, 128, 4096
N_CORES = 8
HQ = H // N_CORES  # 4 q heads per core
COLS = B * HQ * QL  # 256 = (b, h, i) columns of the per-core attention state
THETA = 10000.0
SCALE = 1.0 / math.sqrt(D)
NJMAX = KV // 128  # 32


def _newkey_layout(L):
    """New keys sit at their natural key slots L..L+QL-1: rows rr..rr+3 of
    chunk s0 (straddling into chunk s0+1 when rr > 124). PV-new matmuls need
    a legal base partition (0/32/64), so they run from base32 over the
    zero-padded xv_pad rows. Returns (rr, s0, base32, m) with m = rows of
    the new keys that fit in chunk s0."""
    rr = L % 128
    s0 = L // 128
    if rr >= 64:
        base32 = 64  # window [64, rr+QL) stays within the 64-wide segment
    elif rr >= 32 and rr + QL <= 64:
        base32 = 32  # window may not cross partition 64 from base 32
    else:
        base32 = 0
    m = min(QL, 128 - rr)
    return rr, s0, base32, m


def _install_ntff_hook():
    """Make run_bass_kernel_spmd(trace=True) work in this image: register the
    NTFF profile hook that trn_boot could not (antenv.axon_hooks missing)."""
    try:
        from antenv.axon_hooks import get_axon_ntff_profile_hook  # noqa: F401

        return
    except ImportError:
        pass
    try:
        import antenv
        from trn_agent_boot.trn_boot import _ntff_profile_via_ctypes

        hook = _ntff_profile_via_ctypes("/opt/axon/libaxon_pjrt.so")
        mod = types.ModuleType("antenv.axon_hooks")
        mod.get_axon_ntff_profile_hook = lambda: hook
        mod.set_axon_ntff_profile_hook = lambda h: None
        sys.modules["antenv.axon_hooks"] = mod
        antenv.axon_hooks = mod
    except Exception:
        pass


def _sub_ap(ap, free_dims, extra_offset=0):
    """AP with the same tensor/partition dim but custom free [step, count] dims."""
    import concourse.bass as bass

    return bass.AP(
        tensor=ap.tensor, offset=ap.offset + extra_offset, ap=[ap.ap[0]] + free_dims
    )


def _build(cache_len):
    """Build the per-core Bacc graph, specialized to cache_len (np.int array [B])."""
    import concourse.bacc as bacc
    import concourse.mybir as mybir
    import concourse.tile as tile
    from contextlib import ExitStack

    f32 = mybir.dt.float32
    bf16 = mybir.dt.bfloat16
    f8 = mybir.dt.float8e3
    Exp = mybir.ActivationFunctionType.Exp

    nc = bacc.Bacc("TRN2", target_bir_lowering=False, debug=False, num_devices=N_CORES)

    xT_d = nc.dram_tensor("xT", [128, 32, 64], bf16, kind="ExternalInput").ap()
    wq_d = nc.dram_tensor("wq", [DIM, HQ * D], bf16, kind="ExternalInput").ap()
    wk_d = nc.dram_tensor("wk", [128, 32, D], bf16, kind="ExternalInput").ap()
    wv_d = nc.dram_tensor("wv", [128, 32, D], bf16, kind="ExternalInput").ap()
    wo_d = nc.dram_tensor("wo", [HQ * D, DIM], bf16, kind="ExternalInput").ap()
    # ragged KV, split buffers: K^T in fp8e3 (values pre-scaled x2 on host,
    # folded out via the exp scale), V swizzled in bf16
    L0s_pre = [int(v) for v in cache_len]
    nJs_pre = [(L + 127) // 128 for L in L0s_pre]
    k_bases = []
    v_bases = []
    koff = 0
    voff = 0
    for L, nJ in zip(L0s_pre, nJs_pre):
        Lp = (L + 15) // 16 * 16
        k_bases.append(koff)
        koff += 128 * Lp
        v_bases.append(voff)
        voff += 128 * nJ * 128
    k8_d = nc.dram_tensor("k8", [max(koff, 16)], f8, kind="ExternalInput").ap()
    v_d = nc.dram_tensor("v", [max(voff, 16)], f8, kind="ExternalInput").ap()
    cos_d = nc.dram_tensor("cosb", [B * QL, D // 2], f32, kind="ExternalInput").ap()
    sin_d = nc.dram_tensor("sinb", [B * QL, D // 2], f32, kind="ExternalInput").ap()
    nmask_d = nc.dram_tensor("nmask", [128, B, 32], bf16, kind="ExternalInput").ap()
    ident_d = nc.dram_tensor("ident", [64, 64], f32, kind="ExternalInput").ap()
    out_d = nc.dram_tensor("out", [8, 64, 512], bf16, kind="ExternalOutput").ap()

    L0s = [int(v) for v in cache_len]
    nJs = [(L + 127) // 128 for L in L0s]
    # new keys ride at their natural slots in the key-chunk space
    layouts = [_newkey_layout(L) for L in L0s]  # (rr, s0, base32, m)
    nJxs = [(L + QL + 127) // 128 for L in L0s]
    max_nJ = max(nJxs)

    with tile.TileContext(nc) as tc, ExitStack() as ctx:
        const = ctx.enter_context(tc.tile_pool(name="const", bufs=1))
        wstream = ctx.enter_context(tc.tile_pool(name="wstream", bufs=4))
        ropep = ctx.enter_context(tc.tile_pool(name="ropep", bufs=1))
        kvp = ctx.enter_context(tc.tile_pool(name="kvp", bufs=6))
        probsp = ctx.enter_context(tc.tile_pool(name="probsp", bufs=5))
        fin = ctx.enter_context(tc.tile_pool(name="fin", bufs=1))
        # PSUM pools (8 banks total, stack allocator). The attention pools
        # (psc/pacc/psums/pbc: 3+1+1+1 banks) coexist with the projection
        # pool (2 banks) during the early overlap; py (2) comes after both.
        psB = ctx.enter_context(ExitStack())
        psc = psB.enter_context(tc.tile_pool(name="psc", bufs=3, space="PSUM"))
        pacc = psB.enter_context(tc.tile_pool(name="pacc", bufs=1, space="PSUM"))
        psums = psB.enter_context(tc.tile_pool(name="psums", bufs=1, space="PSUM"))
        pbc = psB.enter_context(tc.tile_pool(name="pbc", bufs=1, space="PSUM"))
        psA = ctx.enter_context(ExitStack())
        pproj = psA.enter_context(tc.tile_pool(name="pproj", bufs=1, space="PSUM"))

        # ---- constants ----
        ident = const.tile([64, 64], f32)
        nc.scalar.dma_start(out=ident, in_=ident_d)
        ones128 = const.tile([128, 1], bf16)
        nc.vector.memset(ones128, 1.0)
        ones_row = const.tile([1, 128], f32)
        nc.vector.memset(ones_row, 1.0)
        cos_sb = const.tile([64, 64], f32)
        nc.scalar.dma_start(out=cos_sb, in_=cos_d)
        sin_sb = const.tile([64, 64], f32)
        nc.scalar.dma_start(out=sin_sb, in_=sin_d)
        # per-b causal masks for the in-stream new keys, partition-aligned to
        # the probs rows they multiply (chunk s0's 16 cols + the next
        # chunk's for the straddle case)
        masks_sb = const.tile([128, B, 32], bf16)
        nc.scalar.dma_start(out=masks_sb, in_=nmask_d)
        xT = const.tile([128, 32, 64], bf16)
        nc.scalar.dma_start(out=xT, in_=xT_d)

        # ---- KV stream DMA emitters (SP ring). Emitted in K0 | wq-half |
        # V0 | K1 V1 | ... order so the first scores are gated only by wq. ----
        import concourse.bass as bass

        k_tiles = {}
        v_tiles = {}
        probs_tiles = {}

        def emit_load_k(b):
            L0 = L0s[b]
            kT_t = kvp.tile([128, max_nJ * 128], f8, tag="kT")
            if L0 > 0:
                Lp = (L0 + 15) // 16 * 16
                src_k = bass.AP(
                    tensor=k8_d.tensor, offset=k_bases[b], ap=[[Lp, 128], [1, L0]]
                )
                nc.sync.dma_start(out=kT_t[:, :L0], in_=src_k)
            k_tiles[b] = kT_t

        def emit_load_v(b):
            nJ = nJs[b]
            v_tt = kvp.tile([128, max_nJ * 128], f8, tag="v")
            if nJ > 0:
                src_v = bass.AP(
                    tensor=v_d.tensor,
                    offset=v_bases[b],
                    ap=[[nJ * 128, 128], [1, nJ * 128]],
                )
                nc.sync.dma_start(out=v_tt[:, : nJ * 128], in_=src_v)
            v_tiles[b] = v_tt[:].rearrange("p (s d) -> p s d", d=D)

        # wk/wv FIRST on the ACT ring: the whole new-key path (kTn_pad /
        # xv_pad) gates every sequence's tail-chunk scores and PV, so it must
        # be ready before the attention pipeline starts flowing
        wk_sb = const.tile([128, 32, D], bf16)
        nc.scalar.dma_start(out=wk_sb, in_=wk_d)
        wv_sb = const.tile([128, 32, D], bf16)
        nc.scalar.dma_start(out=wv_sb, in_=wv_d)

        # ---- projections. wq split across BOTH HWDGE rings so q (which
        # gates all attention) is resident as early as possible ----
        xq_ps = pproj.tile([64, HQ * D], f32)
        xkv_ps = pproj.tile([64, 2, D], f32)  # xk and xv share one PSUM bank
        xk_ps = xkv_ps[:, 0, :]
        xv_ps = xkv_ps[:, 1, :]
        wq_tiles = []
        for g in range(4):
            wq_t = wstream.tile([128, 8, HQ * D], bf16, tag="w", name=f"wq_t{g}")
            ring = nc.scalar if g < 2 else nc.sync
            ring.dma_start(
                out=wq_t,
                in_=wq_d[g * 1024 : (g + 1) * 1024, :].rearrange(
                    "(n p) d -> p n d", p=128
                ),
            )
            wq_tiles.append(wq_t)
            for j in range(8):
                k = g * 8 + j
                nc.tensor.matmul(
                    xq_ps, xT[:, k], wq_t[:, j], start=(k == 0), stop=(k == 31)
                )
        emit_load_k(0)

        # q path: RoPE + transpose to qT immediately (gates all attention)
        q_rope = const.tile([64, HQ * D], f32)
        cosb4 = _sub_ap(cos_sb[:], [[0, HQ], [1, 64]])
        sinb4 = _sub_ap(sin_sb[:], [[0, HQ], [1, 64]])
        q_te = _sub_ap(xq_ps[:], [[128, HQ], [2, 64]])
        q_to = _sub_ap(xq_ps[:], [[128, HQ], [2, 64]], extra_offset=1)
        qr_te = _sub_ap(q_rope[:], [[128, HQ], [2, 64]])
        qr_to = _sub_ap(q_rope[:], [[128, HQ], [2, 64]], extra_offset=1)
        t1 = ropep.tile([64, HQ, 64], f32)
        t2 = ropep.tile([64, HQ, 64], f32)
        t3 = ropep.tile([64, HQ, 64], f32)
        t4 = ropep.tile([64, HQ, 64], f32)
        nc.vector.tensor_mul(t1, q_te, cosb4)
        nc.vector.tensor_mul(t2, q_to, sinb4)
        nc.vector.tensor_sub(qr_te, t1[:], t2[:])
        nc.vector.tensor_mul(t3, q_to, cosb4)
        nc.vector.tensor_mul(t4, q_te, sinb4)
        nc.vector.tensor_add(qr_to, t3[:], t4[:])
        qT = const.tile([128, COLS], bf16)
        for h in range(HQ):
            pt = psc.tile([128, 64], f32, tag="sc")
            nc.tensor.transpose(pt, q_rope[:, h * 128 : (h + 1) * 128], ident)
            qT_dst = _sub_ap(qT[:], [[16, B], [1, QL]], extra_offset=h * QL)
            nc.vector.tensor_copy(
                out=qT_dst, in_=pt[:].rearrange("p (b i) -> p b i", i=QL)
            )

        # prewarm the ACT exp table (after the ACT-ring DMA issues)
        warm = const.tile([1, 1], f32)
        nc.scalar.activation(out=warm, in_=ones_row[0:1, 0:1], func=Exp)

        for k in range(32):
            nc.tensor.matmul(
                xk_ps, xT[:, k], wk_sb[:, k], start=(k == 0), stop=(k == 31)
            )
        for k in range(32):
            nc.tensor.matmul(
                xv_ps, xT[:, k], wv_sb[:, k], start=(k == 0), stop=(k == 31)
            )

        # ---- RoPE (interleaved) on xk; xv plain copy (cast bf16) ----
        k_rope = const.tile([64, D], f32)
        xv_sb = const.tile([64, D], bf16)
        # x2 to match the fp8 V stream's scale (folded out on the host)
        nc.scalar.activation(
            out=xv_sb, in_=xv_ps, func=mybir.ActivationFunctionType.Copy,
            scale=2.0,
        )


        cosb1 = _sub_ap(cos_sb[:], [[1, 64]])
        sinb1 = _sub_ap(sin_sb[:], [[1, 64]])
        k_te = _sub_ap(xk_ps, [[2, 64]])
        k_to = _sub_ap(xk_ps, [[2, 64]], extra_offset=1)
        kr_te = _sub_ap(k_rope[:], [[2, 64]])
        kr_to = _sub_ap(k_rope[:], [[2, 64]], extra_offset=1)
        s1 = ropep.tile([64, 64], f32)
        s2 = ropep.tile([64, 64], f32)
        s3 = ropep.tile([64, 64], f32)
        s4 = ropep.tile([64, 64], f32)
        nc.vector.tensor_mul(s1, k_te, cosb1)
        nc.vector.tensor_mul(s2, k_to, sinb1)
        nc.vector.tensor_sub(kr_te, s1[:], s2[:])
        nc.vector.tensor_mul(s3, k_to, cosb1)
        nc.vector.tensor_mul(s4, k_te, sinb1)
        nc.vector.tensor_add(kr_to, s3[:], s4[:])

        # ---- transpose k_new to [d, cols] (x2 scale folded in so it matches
        # the fp8 K stream's exp scale; cast to fp8 for the in-stream append)
        Copy = mybir.ActivationFunctionType.Copy
        kTn = const.tile([128, 64], bf16)
        pt = psc.tile([128, 64], f32, tag="sc")
        nc.tensor.transpose(pt, k_rope, ident)
        nc.scalar.activation(out=kTn, in_=pt[:], func=Copy, scale=2.0)
        # zero-padded per-b copy: new-key cols at (rr-base32).., so a matmul
        # from base32 writes zeros below the new keys' rows (the old rows
        # there are accumulated on top afterwards)
        kTn_pad = const.tile([128, B, 68], bf16)
        nc.vector.memset(kTn_pad, 0.0)
        for bb in range(B):
            rr_b, _, b32_b, m_b = layouts[bb]
            nc.vector.tensor_copy(
                out=kTn_pad[:, bb, rr_b - b32_b : rr_b - b32_b + m_b],
                in_=kTn[:, bb * QL : bb * QL + m_b],
            )

        # v_new rows placed at the partitions matching their probs rows
        # (rr_b + j, wrapping into the next chunk's rows when straddling),
        # zero elsewhere so the PV-new matmul can run from a legal 32-aligned
        # base over the zero padding. SBUF APs cannot regroup the partition
        # dim, so bounce through DRAM on the gpsimd SWDGE queue, keeping
        # both HWDGE FIFOs clear.
        xv_scratch = nc.dram_tensor("xv_scratch", [B * QL, D], bf16).ap()
        nc.gpsimd.dma_start(out=xv_scratch, in_=xv_sb[:])
        xv_pad = const.tile([128, B, D], bf16)
        nc.vector.memset(xv_pad, 0.0)
        for bb in range(B):
            rr_b, _, _, m_b = layouts[bb]
            nc.gpsimd.dma_start(
                out=xv_pad[rr_b : rr_b + m_b, bb, :],
                in_=xv_scratch[bb * QL : bb * QL + m_b, :],
            )
            if m_b < QL:
                nc.gpsimd.dma_start(
                    out=xv_pad[0 : QL - m_b, bb, :],
                    in_=xv_scratch[bb * QL + m_b : (bb + 1) * QL, :],
                )

        def qT_b(b):
            return qT[:, b * 16 : (b + 1) * 16]

        # phase A PSUM done (x^T, projections, small transposes)
        psA.close()

        # ---- ragged attention, pipelined per sequence. The 4 new keys ride
        # in the tail chunk of the K stream (appended from kTn), so scores,
        # exp, and sums need no separate new-key path; only PV needs a small
        # extra matmul (the V rows for new keys aren't in the V stream). ----
        pv_ps = pacc.tile([128, COLS], f32)
        sums_sb = fin.tile([1, COLS], f32)

        def emit_scores(b):
            L0, nJx = L0s[b], nJxs[b]
            rr, s0, base32, m = layouts[b]
            kT_t = k_tiles[b]
            sc = psc.tile([128, max_nJ * 16], f32, tag="sc")
            qb = qT_b(b)
            if (L0 + QL) % 128:
                # pre-fill the tail chunk's columns with -1e30 so exp() zeroes
                # the unused partitions; the matmuls overwrite the live rows.
                nc.vector.memset(sc[:, (nJx - 1) * 16 : nJx * 16], -1e30)
            for s in range(s0):
                nc.tensor.matmul(
                    sc[0:128, s * 16 : (s + 1) * 16],
                    kT_t[:, s * 128 : (s + 1) * 128],
                    qb,
                    start=True,
                    stop=True,
                )
            # tail chunk: the bf16 new-key scores write first from a legal
            # base (zero-pad rows below), then the old rows land around them
            # (rows [base32, rr) accumulate onto the zeros)
            cs0 = s0 * 16
            nc.tensor.matmul(
                sc[base32 : rr + m, cs0 : cs0 + 16],
                kTn_pad[:, b, 0 : rr + m - base32],
                qb,
                start=True,
                stop=True,
            )
            if base32 > 0:
                nc.tensor.matmul(
                    sc[0:base32, cs0 : cs0 + 16],
                    kT_t[:, s0 * 128 : s0 * 128 + base32],
                    qb,
                    start=True,
                    stop=True,
                )
            if rr > base32:
                nc.tensor.matmul(
                    sc[base32:rr, cs0 : cs0 + 16],
                    kT_t[:, s0 * 128 + base32 : s0 * 128 + rr],
                    qb,
                    start=False,
                    stop=True,
                    skip_group_check=True,
                )
            if m < QL:
                nc.tensor.matmul(
                    sc[0 : QL - m, (s0 + 1) * 16 : (s0 + 2) * 16],
                    kTn[:, b * QL + m : (b + 1) * QL],
                    qb,
                    start=True,
                    stop=True,
                )
            probs = probsp.tile([128, max_nJ * 16], bf16, tag="probs")
            # K was shipped x2 in fp8; fold the 1/2 into the exp scale
            nc.scalar.activation(
                out=probs[:, : nJx * 16], in_=sc[:, : nJx * 16], func=Exp,
                scale=SCALE * 0.5,
            )
            # causal mask on the new-key rows (per-b aligned mask tile; two
            # chunk col-groups when the new keys straddle a chunk boundary)
            w = 16 if m == QL else 32
            pm = probs[:, s0 * 16 : s0 * 16 + w]
            nc.vector.tensor_mul(pm, pm, masks_sb[:, b, :w])
            probs_tiles[b] = probs

        def emit_sums_pv(b):
            L0, nJx, nJ = L0s[b], nJxs[b], nJs[b]
            rr, s0, base32, m = layouts[b]
            c0, c1 = b * 16, (b + 1) * 16
            probs = probs_tiles.get(b)
            v_t = v_tiles.get(b)
            # sums of exp in ONE matmul over all chunks: the stride-0 out AP
            # accumulates each 16-col group onto the same PSUM address
            sums_t = psums.tile([1, 16], f32, tag="sums", name=f"sums{b}")
            sums_acc = _sub_ap(sums_t[:], [[0, nJx], [1, 16]])
            nc.tensor.matmul(
                sums_acc,
                ones128,
                probs[:, : nJx * 16],
                start=True,
                stop=True,
            )
            nc.vector.tensor_copy(out=sums_sb[0:1, c0:c1], in_=sums_t)
            # PV accumulation over the old-V chunks . probs chunks
            for s in range(nJ):
                cj = min(128, L0 - s * 128)
                nc.tensor.matmul(
                    pv_ps[:, c0:c1],
                    v_t[0:cj, s, :],
                    probs[0:cj, s * 16 : (s + 1) * 16],
                    start=(s == 0),
                    stop=False,
                )
            # new-key PV from the zero-padded xv rows, run from a legal
            # 32-aligned base (zero rows below rr contribute nothing)
            nc.tensor.matmul(
                pv_ps[:, c0:c1],
                xv_pad[base32 : rr + m, b, :],
                probs[base32 : rr + m, s0 * 16 : (s0 + 1) * 16],
                start=(nJ == 0),
                stop=(m == QL),
            )
            if m < QL:
                nc.tensor.matmul(
                    pv_ps[:, c0:c1],
                    xv_pad[0 : QL - m, b, :],
                    probs[0 : QL - m, (s0 + 1) * 16 : (s0 + 2) * 16],
                    start=False,
                    stop=True,
                )

        # wo weights prefetch on the ACT HWDGE ring; issued mid-stream (at
        # b==10 below) so the early HBM bandwidth goes to the KV stream
        wo_tiles = []

        def emit_wo_prefetch():
            for h in range(HQ):
                wo_t = wstream.tile(
                    [128, 8, 512], bf16, tag="wo", name=f"wo_t{h}"
                )
                nc.scalar.dma_start(
                    out=wo_t,
                    in_=wo_d[h * 128 : (h + 1) * 128, :].rearrange(
                        "p (n d) -> p n d", d=512
                    ),
                )
                wo_tiles.append(wo_t)

        # ---- finalize: attnT = pv / sums (per half, so the first half
        # overlaps the second half's attention stream) ----
        # attnT in h-major cols (h*64 + b*4 + i) so the wo matmul lhsT is a
        # contiguous [128, 64] slice; the divide does the (b,h) permute.
        attnT = fin.tile([128, COLS], bf16)

        def emit_finalize_group(b0, nb, gi):
            c0 = b0 * 16
            w = nb * 16
            bc_ps = pbc.tile([128, 128], f32, tag="bc", name=f"bc{gi}")
            nc.tensor.matmul(
                bc_ps[:, :w], ones_row, sums_sb[0:1, c0 : c0 + w],
                start=True, stop=True,
            )
            # reciprocal after the broadcast: 128 lanes instead of 1
            bc_sb = fin.tile([128, 128], f32, tag="bc_sb", name=f"bc_sb{gi}")
            nc.vector.reciprocal(out=bc_sb[:, :w], in_=bc_ps[:, :w])
            attnT_dst = _sub_ap(
                attnT[:], [[4, nb], [64, HQ], [1, QL]], extra_offset=b0 * 4
            )
            nc.vector.tensor_mul(
                attnT_dst,
                _sub_ap(pv_ps[:], [[16, nb], [4, HQ], [1, QL]], extra_offset=c0),
                _sub_ap(bc_sb[:], [[16, nb], [4, HQ], [1, QL]]),
            )


        def attnT_h(h):
            return attnT[:, h * 64 : (h + 1) * 64]

        emit_load_v(0)
        for b in range(B):
            if b + 1 < B:
                emit_load_k(b + 1)
                emit_load_v(b + 1)
            emit_scores(b)
            if b > 1:
                emit_sums_pv(b - 2)
            if b == 9:
                emit_finalize_group(0, 8, 0)
            if b == 6:
                emit_wo_prefetch()
            if b == 13:
                emit_finalize_group(8, 4, 1)
        emit_sums_pv(B - 2)
        emit_finalize_group(12, 2, 2)
        emit_sums_pv(B - 1)
        emit_finalize_group(14, 2, 3)


        # phase B PSUM done (attention)
        psB.close()
        py = ctx.enter_context(tc.tile_pool(name="py", bufs=2, space="PSUM"))

        # ---- output projection: y[64, 4096] = attn[64, 512] @ wo ----
        # out is written bf16, bank-contiguous ([8][64, 512]); host reassembles
        y_sb = fin.tile([64, DIM], bf16)
        y_banks = [
            py.tile([64, 512], f32, tag="y", name=f"y_bank{nt}")
            for nt in range(8)
        ]
        for nt in range(8):
            for h in range(HQ):
                nc.tensor.matmul(
                    y_banks[nt],
                    attnT_h(h),
                    wo_tiles[h][:, nt, :],
                    start=(h == 0),
                    stop=(h == HQ - 1),
                )
            # alternate PSUM->SBUF drain between DVE and ACT so the two copy
            # pipes overlap the next bank's matmuls
            if nt % 2 == 0:
                nc.vector.tensor_copy(
                    out=y_sb[:, nt * 512 : (nt + 1) * 512], in_=y_banks[nt]
                )
            else:
                nc.scalar.activation(
                    out=y_sb[:, nt * 512 : (nt + 1) * 512], in_=y_banks[nt],
                    func=Copy,
                )
            nc.scalar.dma_start(
                out=out_d[nt],
                in_=y_sb[:, nt * 512 : (nt + 1) * 512],
            )

    nc.compile()
    return nc


_CACHE = {}


def _get_nc(cache_len):
    key = tuple(int(v) for v in cache_len)
    if key not in _CACHE:
        _CACHE[key] = _build(cache_len)
    return _CACHE[key]


def _prep_shards(x, wq, wk, wv, wo, cache_k, cache_v, cache_len):
    import concourse.mybir as mybir

    bf16 = mybir.dt.np(mybir.dt.bfloat16)

    cache_len = np.asarray(cache_len, dtype=np.int32)
    # sort sequences by descending live length: big sequences stream first,
    # small ones land in the drain window; host unpermutes the output rows
    perm = np.argsort(-cache_len, kind="stable")
    cache_len = cache_len[perm]
    x = np.ascontiguousarray(
        np.asarray(x, dtype=np.float32).reshape(B, QL, DIM)[perm].reshape(B * QL, DIM)
    )
    cache_k = cache_k[perm]
    cache_v = cache_v[perm]
    L0s = [int(v) for v in cache_len]
    nJs = [(L + 127) // 128 for L in L0s]

    pos = (cache_len[:, None].astype(np.int64) + np.arange(QL)[None, :]).reshape(-1)
    inv_freq = 1.0 / (THETA ** (np.arange(D // 2, dtype=np.float64) / (D // 2)))
    ang = pos[:, None] * inv_freq[None, :]
    cosb = np.cos(ang).astype(np.float32)
    sinb = np.sin(ang).astype(np.float32)

    # per-b causal masks for the in-stream new keys: new key j sits at probs
    # row RR+j of its chunk; cols are that chunk's 16 (h,i) cols. 1.0 rows
    # elsewhere (gap rows were already -1e30 in the scores).
    nmask = np.ones((128, B, 32), dtype=np.float32)
    for b in range(B):
        rr, _, _, _ = _newkey_layout(L0s[b])
        for j in range(QL):
            p, blk = rr + j, 0
            if p >= 128:
                p, blk = p - 128, 1
            for c in range(16):
                if c % QL < j:
                    nmask[p, b, blk * 16 + c] = 0.0
    nmask = nmask.astype(bf16)

    # K^T per kv-head: [KVH, B, D, KV] in fp8e3 (x2 scale, clipped to the
    # e3m4 range; the device folds the 1/2 into the exp scale). V swizzled
    # bf16 so DMA runs stay long: v_all[c, b, p, s, d] = V[c, b, s*128+p, d]
    f8np = mybir.dt.np(mybir.dt.float8e3)
    kT_all = np.clip(
        np.ascontiguousarray(np.transpose(cache_k, (2, 0, 3, 1))) * 2.0,
        -15.5, 15.5,
    ).astype(f8np)
    v_all = np.clip(
        np.ascontiguousarray(
            np.transpose(
                cache_v.reshape(B, NJMAX, 128, KVH, D), (3, 0, 2, 1, 4)
            )
        )
        * 2.0,
        -15.5, 15.5,
    ).astype(f8np)  # [KVH, B, 128, NJMAX, D], fp8 x2 like the K stream
    k_bases = []
    v_bases = []
    koff = 0
    voff = 0
    for L, nJ in zip(L0s, nJs):
        Lp = (L + 15) // 16 * 16
        k_bases.append(koff)
        koff += 128 * Lp
        v_bases.append(voff)
        voff += 128 * nJ * 128
    k_total = max(koff, 16)
    v_total = max(voff, 16)

    def pack_k8(c):
        buf = np.zeros(k_total, dtype=f8np)
        for b in range(B):
            L = L0s[b]
            if L == 0:
                continue
            Lp = (L + 15) // 16 * 16
            block = buf[k_bases[b] : k_bases[b] + 128 * Lp].reshape(128, Lp)
            block[:, :L] = kT_all[c, b, :, :L]
        return buf

    def pack_v(c):
        buf = np.zeros(v_total, dtype=f8np)
        for b in range(B):
            nJ = nJs[b]
            if nJ == 0:
                continue
            block = buf[v_bases[b] : v_bases[b] + 128 * nJ * 128].reshape(
                128, nJ * 128
            )
            block[:] = v_all[c, b, :, :nJ, :].reshape(128, nJ * D)
        return buf

    xT_host = np.ascontiguousarray(
        x.T.reshape(32, 128, 64).transpose(1, 0, 2)
    ).astype(bf16)

    in_maps = []
    for c in range(N_CORES):
        wk_c = wk[:, c * 128 : (c + 1) * 128].reshape(32, 128, 128)
        wv_c = wv[:, c * 128 : (c + 1) * 128].reshape(32, 128, 128)
        in_maps.append(
            {
                "xT": xT_host,
                "wq": np.ascontiguousarray(wq[:, c * 512 : (c + 1) * 512]).astype(
                    bf16
                ),
                "wk": np.ascontiguousarray(np.transpose(wk_c, (1, 0, 2))).astype(bf16),
                "wv": np.ascontiguousarray(np.transpose(wv_c, (1, 0, 2))).astype(bf16),
                "wo": np.ascontiguousarray(wo[c * 512 : (c + 1) * 512, :]).astype(
                    bf16
                ),
                "k8": pack_k8(c),
                "v": pack_v(c),
                "cosb": cosb,
                "sinb": sinb,
                "nmask": nmask,
                "ident": np.eye(64, dtype=np.float32),
            }
        )
    return in_maps, cache_len, perm


def _run(inputs, trace=False, trace_kwargs=None):
    _install_ntff_hook()
    from concourse.bass_utils import run_bass_kernel_spmd

    in_maps, cache_len, perm = _prep_shards(**inputs)
    nc = _get_nc(cache_len)
    res = run_bass_kernel_spmd(
        nc,
        in_maps,
        core_ids=list(range(N_CORES)),
        trace=trace,
        **(trace_kwargs or {}),
    )
    out_p = np.zeros((B * QL, DIM), dtype=np.float32)
    for i in range(N_CORES):
        ob = np.asarray(res.results[i]["out"], dtype=np.float32)  # [8, 64, 512]
        out_p += np.transpose(ob, (1, 0, 2)).reshape(B * QL, DIM)
    out_p *= 0.5  # V stream (and v_new) carry a x2 scale
    out = np.zeros_like(out_p)
    out.reshape(B, QL, DIM)[perm] = out_p.reshape(B, QL, DIM)
    return out, res


def kernel(**inputs):
    out, _ = _run(inputs, trace=False)
    return out


def kernel_profiled(**inputs):
    out, res = _run(inputs, trace=True)
    return out, res



# revision 53
# speedup vs baseline: 1.1212x; 1.1212x over previous
"""Trainium2 Bass kernel for ragged GQA attention decode (B=16, QL=4, KV=4096,
H=32, KVH=8, D=128, DIM=4096), tensor-parallel over 8 NeuronCores.

Sharding: core c owns q-heads [4c, 4c+4) and kv-head c. wq/wk/wv are
column-split, wo row-split, KV cache split along the kv-head dim. Each core
computes a partial [64, 4096] output (its heads through its wo rows); the
host sums the 8 partials.

The Bass graph is specialized to the actual cache_len values (known on host
at build time), so only the live prefix of the KV cache is ever read.

Compute runs in bf16 (f32 PSUM accumulation): the weights and KV cache are
shipped to the device as bf16 shards, halving HBM traffic and making the
TensorEngine matmuls single-pass.
"""

import math
import sys
import types

import numpy as np

B, QL, KV, H, KVH, D, DIM = 16, 4, 4096, 32, 8, 128, 4096
N_CORES = 8
HQ = H // N_CORES  # 4 q heads per core
COLS = B * HQ * QL  # 256 = (b, h, i) columns of the per-core attention state
THETA = 10000.0
SCALE = 1.0 / math.sqrt(D)
NJMAX = KV // 128  # 32


def _newkey_layout(L):
    """New keys sit at their natural key slots L..L+QL-1: rows rr..rr+3 of
    chunk s0 (straddling into chunk s0+1 when rr > 124). PV-new matmuls need
    a legal base partition (0/32/64), so they run from base32 over the
    zero-padded xv_pad rows. Returns (rr, s0, base32, m) with m = rows of
    the new keys that fit in chunk s0."""
    rr = L % 128
    s0 = L // 128
    if rr >= 64:
        base32 = 64  # window [64, rr+QL) stays within the 64-wide segment
    elif rr >= 32 and rr + QL <= 64:
        base32 = 32  # window may not cross partition 64 from base 32
    else:
        base32 = 0
    m = min(QL, 128 - rr)
    return rr, s0, base32, m


def _install_ntff_hook():
    """Make run_bass_kernel_spmd(trace=True) work in this image: register the
    NTFF profile hook that trn_boot could not (antenv.axon_hooks missing)."""
    try:
        from antenv.axon_hooks import get_axon_ntff_profile_hook  # noqa: F401

        return
    except ImportError:
        pass
    try:
        import antenv
        from trn_agent_boot.trn_boot import _ntff_profile_via_ctypes

        hook = _ntff_profile_via_ctypes("/opt/axon/libaxon_pjrt.so")
        mod = types.ModuleType("antenv.axon_hooks")
        mod.get_axon_ntff_profile_hook = lambda: hook
        mod.set_axon_ntff_profile_hook = lambda h: None
        sys.modules["antenv.axon_hooks"] = mod
        antenv.axon_hooks = mod
    except Exception:
        pass


def _sub_ap(ap, free_dims, extra_offset=0):
    """AP with the same tensor/partition dim but custom free [step, count] dims."""
    import concourse.bass as bass

    return bass.AP(
        tensor=ap.tensor, offset=ap.offset + extra_offset, ap=[ap.ap[0]] + free_dims
    )


def _build(cache_len):
    """Build the per-core Bacc graph, specialized to cache_len (np.int array [B])."""
    import concourse.bacc as bacc
    import concourse.mybir as mybir
    import concourse.tile as tile
    from contextlib import ExitStack

    f32 = mybir.dt.float32
    bf16 = mybir.dt.bfloat16
    f8 = mybir.dt.float8e3
    Exp = mybir.ActivationFunctionType.Exp

    nc = bacc.Bacc("TRN2", target_bir_lowering=False, debug=False, num_devices=N_CORES)

    xT_d = nc.dram_tensor("xT", [128, 32, 64], bf16, kind="ExternalInput").ap()
    wq_d = nc.dram_tensor("wq", [DIM, HQ * D], bf16, kind="ExternalInput").ap()
    wk_d = nc.dram_tensor("wk", [128, 32, D], bf16, kind="ExternalInput").ap()
    wv_d = nc.dram_tensor("wv", [128, 32, D], bf16, kind="ExternalInput").ap()
    wo_d = nc.dram_tensor("wo", [HQ * D, DIM], bf16, kind="ExternalInput").ap()
    # ragged KV, split buffers: K^T in fp8e3 (values pre-scaled x2 on host,
    # folded out via the exp scale), V swizzled in bf16
    L0s_pre = [int(v) for v in cache_len]
    nJs_pre = [(L + 127) // 128 for L in L0s_pre]
    k_bases = []
    v_bases = []
    koff = 0
    voff = 0
    for L, nJ in zip(L0s_pre, nJs_pre):
        Lp = (L + 15) // 16 * 16
        k_bases.append(koff)
        koff += 128 * Lp
        v_bases.append(voff)
        voff += 128 * nJ * 128
    k8_d = nc.dram_tensor("k8", [max(koff, 16)], f8, kind="ExternalInput").ap()
    v_d = nc.dram_tensor("v", [max(voff, 16)], f8, kind="ExternalInput").ap()
    cos_d = nc.dram_tensor("cosb", [B * QL, D // 2], f32, kind="ExternalInput").ap()
    sin_d = nc.dram_tensor("sinb", [B * QL, D // 2], f32, kind="ExternalInput").ap()
    nmask_d = nc.dram_tensor("nmask", [128, B, 32], bf16, kind="ExternalInput").ap()
    ident_d = nc.dram_tensor("ident", [64, 64], f32, kind="ExternalInput").ap()
    out_d = nc.dram_tensor("out", [8, 64, 512], bf16, kind="ExternalOutput").ap()

    L0s = [int(v) for v in cache_len]
    nJs = [(L + 127) // 128 for L in L0s]
    # new keys ride at their natural slots in the key-chunk space
    layouts = [_newkey_layout(L) for L in L0s]  # (rr, s0, base32, m)
    nJxs = [(L + QL + 127) // 128 for L in L0s]
    max_nJ = max(nJxs)

    with tile.TileContext(nc) as tc, ExitStack() as ctx:
        const = ctx.enter_context(tc.tile_pool(name="const", bufs=1))
        wstream = ctx.enter_context(tc.tile_pool(name="wstream", bufs=4))
        ropep = ctx.enter_context(tc.tile_pool(name="ropep", bufs=1))
        kvp = ctx.enter_context(tc.tile_pool(name="kvp", bufs=8))
        probsp = ctx.enter_context(tc.tile_pool(name="probsp", bufs=6))
        fin = ctx.enter_context(tc.tile_pool(name="fin", bufs=1))
        # PSUM pools (8 banks total, stack allocator). The attention pools
        # (psc/pacc/psums/pbc: 3+1+1+1 banks) coexist with the projection
        # pool (2 banks) during the early overlap; py (2) comes after both.
        psB = ctx.enter_context(ExitStack())
        psc = psB.enter_context(tc.tile_pool(name="psc", bufs=3, space="PSUM"))
        pacc = psB.enter_context(tc.tile_pool(name="pacc", bufs=1, space="PSUM"))
        psums = psB.enter_context(tc.tile_pool(name="psums", bufs=1, space="PSUM"))
        pbc = psB.enter_context(tc.tile_pool(name="pbc", bufs=1, space="PSUM"))
        psA = ctx.enter_context(ExitStack())
        pproj = psA.enter_context(tc.tile_pool(name="pproj", bufs=1, space="PSUM"))

        # ---- constants ----
        ident = const.tile([64, 64], f32)
        nc.scalar.dma_start(out=ident, in_=ident_d)
        ones128 = const.tile([128, 1], bf16)
        nc.vector.memset(ones128, 1.0)
        ones_row = const.tile([1, 128], f32)
        nc.vector.memset(ones_row, 1.0)
        cos_sb = const.tile([64, 64], f32)
        nc.scalar.dma_start(out=cos_sb, in_=cos_d)
        sin_sb = const.tile([64, 64], f32)
        nc.scalar.dma_start(out=sin_sb, in_=sin_d)
        # per-b causal masks for the in-stream new keys, partition-aligned to
        # the probs rows they multiply (chunk s0's 16 cols + the next
        # chunk's for the straddle case)
        masks_sb = const.tile([128, B, 32], bf16)
        nc.scalar.dma_start(out=masks_sb, in_=nmask_d)
        xT = const.tile([128, 32, 64], bf16)
        nc.scalar.dma_start(out=xT, in_=xT_d)

        # ---- KV stream DMA emitters (SP ring). Emitted in K0 | wq-half |
        # V0 | K1 V1 | ... order so the first scores are gated only by wq. ----
        import concourse.bass as bass

        k_tiles = {}
        v_tiles = {}
        probs_tiles = {}

        def emit_load_k(b):
            L0 = L0s[b]
            kT_t = kvp.tile([128, max_nJ * 128], f8, tag="kT")
            if L0 > 0:
                Lp = (L0 + 15) // 16 * 16
                src_k = bass.AP(
                    tensor=k8_d.tensor, offset=k_bases[b], ap=[[Lp, 128], [1, L0]]
                )
                nc.sync.dma_start(out=kT_t[:, :L0], in_=src_k)
            k_tiles[b] = kT_t

        def emit_load_v(b):
            nJ = nJs[b]
            v_tt = kvp.tile([128, max_nJ * 128], f8, tag="v")
            if nJ > 0:
                src_v = bass.AP(
                    tensor=v_d.tensor,
                    offset=v_bases[b],
                    ap=[[nJ * 128, 128], [1, nJ * 128]],
                )
                nc.sync.dma_start(out=v_tt[:, : nJ * 128], in_=src_v)
            v_tiles[b] = v_tt[:].rearrange("p (s d) -> p s d", d=D)

        emit_load_k(0)

        # ---- projections. wq split across BOTH HWDGE rings so q (which
        # gates all attention) is resident as early as possible ----
        xq_ps = pproj.tile([64, HQ * D], f32)
        xkv_ps = pproj.tile([64, 2, D], f32)  # xk and xv share one PSUM bank
        xk_ps = xkv_ps[:, 0, :]
        xv_ps = xkv_ps[:, 1, :]
        wq_tiles = []
        for g in range(4):
            wq_t = wstream.tile([128, 8, HQ * D], bf16, tag="w", name=f"wq_t{g}")
            ring = nc.scalar if g < 2 else nc.sync
            ring.dma_start(
                out=wq_t,
                in_=wq_d[g * 1024 : (g + 1) * 1024, :].rearrange(
                    "(n p) d -> p n d", p=128
                ),
            )
            wq_tiles.append(wq_t)
            for j in range(8):
                k = g * 8 + j
                nc.tensor.matmul(
                    xq_ps, xT[:, k], wq_t[:, j], start=(k == 0), stop=(k == 31)
                )

        # q path: RoPE + transpose to qT immediately (gates all attention)
        q_rope = const.tile([64, HQ * D], f32)
        cosb4 = _sub_ap(cos_sb[:], [[0, HQ], [1, 64]])
        sinb4 = _sub_ap(sin_sb[:], [[0, HQ], [1, 64]])
        q_te = _sub_ap(xq_ps[:], [[128, HQ], [2, 64]])
        q_to = _sub_ap(xq_ps[:], [[128, HQ], [2, 64]], extra_offset=1)
        qr_te = _sub_ap(q_rope[:], [[128, HQ], [2, 64]])
        qr_to = _sub_ap(q_rope[:], [[128, HQ], [2, 64]], extra_offset=1)
        t1 = ropep.tile([64, HQ, 64], f32)
        t2 = ropep.tile([64, HQ, 64], f32)
        t3 = ropep.tile([64, HQ, 64], f32)
        t4 = ropep.tile([64, HQ, 64], f32)
        nc.vector.tensor_mul(t1, q_te, cosb4)
        nc.vector.tensor_mul(t2, q_to, sinb4)
        nc.vector.tensor_sub(qr_te, t1[:], t2[:])
        nc.vector.tensor_mul(t3, q_to, cosb4)
        nc.vector.tensor_mul(t4, q_te, sinb4)
        nc.vector.tensor_add(qr_to, t3[:], t4[:])
        qT = const.tile([128, COLS], bf16)
        for h in range(HQ):
            pt = psc.tile([128, 64], f32, tag="sc")
            nc.tensor.transpose(pt, q_rope[:, h * 128 : (h + 1) * 128], ident)
            qT_dst = _sub_ap(qT[:], [[16, B], [1, QL]], extra_offset=h * QL)
            nc.vector.tensor_copy(
                out=qT_dst, in_=pt[:].rearrange("p (b i) -> p b i", i=QL)
            )

        # wk/wv on the ACT ring (behind the wq half) so the SP ring carries
        # only the KV stream after its wq half
        wk_sb = const.tile([128, 32, D], bf16)
        nc.scalar.dma_start(out=wk_sb, in_=wk_d)
        wv_sb = const.tile([128, 32, D], bf16)
        nc.scalar.dma_start(out=wv_sb, in_=wv_d)

        # prewarm the ACT exp table (after the ACT-ring DMA issues)
        warm = const.tile([1, 1], f32)
        nc.scalar.activation(out=warm, in_=ones_row[0:1, 0:1], func=Exp)

        for k in range(32):
            nc.tensor.matmul(
                xk_ps, xT[:, k], wk_sb[:, k], start=(k == 0), stop=(k == 31)
            )
        for k in range(32):
            nc.tensor.matmul(
                xv_ps, xT[:, k], wv_sb[:, k], start=(k == 0), stop=(k == 31)
            )

        # ---- RoPE (interleaved) on xk; xv plain copy (cast bf16) ----
        k_rope = const.tile([64, D], f32)
        xv_sb = const.tile([64, D], bf16)
        # x2 to match the fp8 V stream's scale (folded out on the host)
        nc.scalar.activation(
            out=xv_sb, in_=xv_ps, func=mybir.ActivationFunctionType.Copy,
            scale=2.0,
        )


        cosb1 = _sub_ap(cos_sb[:], [[1, 64]])
        sinb1 = _sub_ap(sin_sb[:], [[1, 64]])
        k_te = _sub_ap(xk_ps, [[2, 64]])
        k_to = _sub_ap(xk_ps, [[2, 64]], extra_offset=1)
        kr_te = _sub_ap(k_rope[:], [[2, 64]])
        kr_to = _sub_ap(k_rope[:], [[2, 64]], extra_offset=1)
        s1 = ropep.tile([64, 64], f32)
        s2 = ropep.tile([64, 64], f32)
        s3 = ropep.tile([64, 64], f32)
        s4 = ropep.tile([64, 64], f32)
        nc.vector.tensor_mul(s1, k_te, cosb1)
        nc.vector.tensor_mul(s2, k_to, sinb1)
        nc.vector.tensor_sub(kr_te, s1[:], s2[:])
        nc.vector.tensor_mul(s3, k_to, cosb1)
        nc.vector.tensor_mul(s4, k_te, sinb1)
        nc.vector.tensor_add(kr_to, s3[:], s4[:])

        # ---- transpose k_new to [d, cols] (x2 scale folded in so it matches
        # the fp8 K stream's exp scale; cast to fp8 for the in-stream append)
        Copy = mybir.ActivationFunctionType.Copy
        kTn = const.tile([128, 64], bf16)
        pt = psc.tile([128, 64], f32, tag="sc")
        nc.tensor.transpose(pt, k_rope, ident)
        nc.scalar.activation(out=kTn, in_=pt[:], func=Copy, scale=2.0)
        # zero-padded per-b copy: new-key cols at (rr-base32).., so a matmul
        # from base32 writes zeros below the new keys' rows (the old rows
        # there are accumulated on top afterwards)
        kTn_pad = const.tile([128, B, 68], bf16)
        nc.vector.memset(kTn_pad, 0.0)
        for bb in range(B):
            rr_b, _, b32_b, m_b = layouts[bb]
            nc.vector.tensor_copy(
                out=kTn_pad[:, bb, rr_b - b32_b : rr_b - b32_b + m_b],
                in_=kTn[:, bb * QL : bb * QL + m_b],
            )

        # v_new rows placed at the partitions matching their probs rows
        # (rr_b + j, wrapping into the next chunk's rows when straddling),
        # zero elsewhere so the PV-new matmul can run from a legal 32-aligned
        # base over the zero padding. SBUF APs cannot regroup the partition
        # dim, so bounce through DRAM on the gpsimd SWDGE queue, keeping
        # both HWDGE FIFOs clear.
        xv_scratch = nc.dram_tensor("xv_scratch", [B * QL, D], bf16).ap()
        nc.gpsimd.dma_start(out=xv_scratch, in_=xv_sb[:])
        xv_pad = const.tile([128, B, D], bf16)
        nc.vector.memset(xv_pad, 0.0)
        for bb in range(B):
            rr_b, _, _, m_b = layouts[bb]
            nc.gpsimd.dma_start(
                out=xv_pad[rr_b : rr_b + m_b, bb, :],
                in_=xv_scratch[bb * QL : bb * QL + m_b, :],
            )
            if m_b < QL:
                nc.gpsimd.dma_start(
                    out=xv_pad[0 : QL - m_b, bb, :],
                    in_=xv_scratch[bb * QL + m_b : (bb + 1) * QL, :],
                )

        def qT_b(b):
            return qT[:, b * 16 : (b + 1) * 16]

        # phase A PSUM done (x^T, projections, small transposes)
        psA.close()

        # ---- ragged attention, pipelined per sequence. The 4 new keys ride
        # in the tail chunk of the K stream (appended from kTn), so scores,
        # exp, and sums need no separate new-key path; only PV needs a small
        # extra matmul (the V rows for new keys aren't in the V stream). ----
        pv_ps = pacc.tile([128, COLS], f32)
        sums_sb = fin.tile([1, COLS], f32)

        def emit_scores(b):
            L0, nJx = L0s[b], nJxs[b]
            rr, s0, base32, m = layouts[b]
            kT_t = k_tiles[b]
            sc = psc.tile([128, max_nJ * 16], f32, tag="sc")
            qb = qT_b(b)
            if (L0 + QL) % 128:
                # pre-fill the tail chunk's columns with -1e30 so exp() zeroes
                # the unused partitions; the matmuls overwrite the live rows.
                nc.vector.memset(sc[:, (nJx - 1) * 16 : nJx * 16], -1e30)
            for s in range(s0):
                nc.tensor.matmul(
                    sc[0:128, s * 16 : (s + 1) * 16],
                    kT_t[:, s * 128 : (s + 1) * 128],
                    qb,
                    start=True,
                    stop=True,
                )
            # tail chunk: the bf16 new-key scores write first from a legal
            # base (zero-pad rows below), then the old rows land around them
            # (rows [base32, rr) accumulate onto the zeros)
            cs0 = s0 * 16
            nc.tensor.matmul(
                sc[base32 : rr + m, cs0 : cs0 + 16],
                kTn_pad[:, b, 0 : rr + m - base32],
                qb,
                start=True,
                stop=True,
            )
            if base32 > 0:
                nc.tensor.matmul(
                    sc[0:base32, cs0 : cs0 + 16],
                    kT_t[:, s0 * 128 : s0 * 128 + base32],
                    qb,
                    start=True,
                    stop=True,
                )
            if rr > base32:
                nc.tensor.matmul(
                    sc[base32:rr, cs0 : cs0 + 16],
                    kT_t[:, s0 * 128 + base32 : s0 * 128 + rr],
                    qb,
                    start=False,
                    stop=True,
                    skip_group_check=True,
                )
            if m < QL:
                nc.tensor.matmul(
                    sc[0 : QL - m, (s0 + 1) * 16 : (s0 + 2) * 16],
                    kTn[:, b * QL + m : (b + 1) * QL],
                    qb,
                    start=True,
                    stop=True,
                )
            probs = probsp.tile([128, max_nJ * 16], bf16, tag="probs")
            # K was shipped x2 in fp8; fold the 1/2 into the exp scale
            nc.scalar.activation(
                out=probs[:, : nJx * 16], in_=sc[:, : nJx * 16], func=Exp,
                scale=SCALE * 0.5,
            )
            # causal mask on the new-key rows (per-b aligned mask tile; two
            # chunk col-groups when the new keys straddle a chunk boundary)
            w = 16 if m == QL else 32
            pm = probs[:, s0 * 16 : s0 * 16 + w]
            nc.vector.tensor_mul(pm, pm, masks_sb[:, b, :w])
            probs_tiles[b] = probs

        def emit_sums_pv(b):
            L0, nJx, nJ = L0s[b], nJxs[b], nJs[b]
            rr, s0, base32, m = layouts[b]
            c0, c1 = b * 16, (b + 1) * 16
            probs = probs_tiles.get(b)
            v_t = v_tiles.get(b)
            # sums of exp in ONE matmul over all chunks: the stride-0 out AP
            # accumulates each 16-col group onto the same PSUM address
            sums_t = psums.tile([1, 16], f32, tag="sums", name=f"sums{b}")
            sums_acc = _sub_ap(sums_t[:], [[0, nJx], [1, 16]])
            nc.tensor.matmul(
                sums_acc,
                ones128,
                probs[:, : nJx * 16],
                start=True,
                stop=True,
            )
            nc.vector.tensor_copy(out=sums_sb[0:1, c0:c1], in_=sums_t)
            # PV accumulation over the old-V chunks . probs chunks
            for s in range(nJ):
                cj = min(128, L0 - s * 128)
                nc.tensor.matmul(
                    pv_ps[:, c0:c1],
                    v_t[0:cj, s, :],
                    probs[0:cj, s * 16 : (s + 1) * 16],
                    start=(s == 0),
                    stop=False,
                )
            # new-key PV from the zero-padded xv rows, run from a legal
            # 32-aligned base (zero rows below rr contribute nothing)
            nc.tensor.matmul(
                pv_ps[:, c0:c1],
                xv_pad[base32 : rr + m, b, :],
                probs[base32 : rr + m, s0 * 16 : (s0 + 1) * 16],
                start=(nJ == 0),
                stop=(m == QL),
            )
            if m < QL:
                nc.tensor.matmul(
                    pv_ps[:, c0:c1],
                    xv_pad[0 : QL - m, b, :],
                    probs[0 : QL - m, (s0 + 1) * 16 : (s0 + 2) * 16],
                    start=False,
                    stop=True,
                )

        # wo weights prefetch on the ACT HWDGE ring; issued mid-stream (at
        # b==10 below) so the early HBM bandwidth goes to the KV stream
        wo_tiles = []

        def emit_wo_prefetch():
            for h in range(HQ):
                wo_t = wstream.tile(
                    [128, 8, 512], bf16, tag="wo", name=f"wo_t{h}"
                )
                nc.scalar.dma_start(
                    out=wo_t,
                    in_=wo_d[h * 128 : (h + 1) * 128, :].rearrange(
                        "p (n d) -> p n d", d=512
                    ),
                )
                wo_tiles.append(wo_t)

        # ---- finalize: attnT = pv / sums (per half, so the first half
        # overlaps the second half's attention stream) ----
        # attnT in h-major cols (h*64 + b*4 + i) so the wo matmul lhsT is a
        # contiguous [128, 64] slice; the divide does the (b,h) permute.
        attnT = fin.tile([128, COLS], bf16)

        def emit_finalize_group(b0, nb, gi):
            c0 = b0 * 16
            w = nb * 16
            bc_ps = pbc.tile([128, 128], f32, tag="bc", name=f"bc{gi}")
            nc.tensor.matmul(
                bc_ps[:, :w], ones_row, sums_sb[0:1, c0 : c0 + w],
                start=True, stop=True,
            )
            # reciprocal after the broadcast: 128 lanes instead of 1
            bc_sb = fin.tile([128, 128], f32, tag="bc_sb", name=f"bc_sb{gi}")
            nc.vector.reciprocal(out=bc_sb[:, :w], in_=bc_ps[:, :w])
            attnT_dst = _sub_ap(
                attnT[:], [[4, nb], [64, HQ], [1, QL]], extra_offset=b0 * 4
            )
            nc.vector.tensor_mul(
                attnT_dst,
                _sub_ap(pv_ps[:], [[16, nb], [4, HQ], [1, QL]], extra_offset=c0),
                _sub_ap(bc_sb[:], [[16, nb], [4, HQ], [1, QL]]),
            )


        def attnT_h(h):
            return attnT[:, h * 64 : (h + 1) * 64]

        emit_load_v(0)
        for b in range(B):
            if b + 1 < B:
                emit_load_k(b + 1)
                emit_load_v(b + 1)
            emit_scores(b)
            if b > 1:
                emit_sums_pv(b - 2)
            if b == 9:
                emit_finalize_group(0, 8, 0)
            if b == 6:
                emit_wo_prefetch()
            if b == 13:
                emit_finalize_group(8, 4, 1)
        emit_sums_pv(B - 2)
        emit_finalize_group(12, 2, 2)
        emit_sums_pv(B - 1)
        emit_finalize_group(14, 2, 3)


        # phase B PSUM done (attention)
        psB.close()
        py = ctx.enter_context(tc.tile_pool(name="py", bufs=2, space="PSUM"))

        # ---- output projection: y[64, 4096] = attn[64, 512] @ wo ----
        # out is written bf16, bank-contiguous ([8][64, 512]); host reassembles
        y_sb = fin.tile([64, DIM], bf16)
        y_banks = [
            py.tile([64, 512], f32, tag="y", name=f"y_bank{nt}")
            for nt in range(8)
        ]
        for nt in range(8):
            for h in range(HQ):
                nc.tensor.matmul(
                    y_banks[nt],
                    attnT_h(h),
                    wo_tiles[h][:, nt, :],
                    start=(h == 0),
                    stop=(h == HQ - 1),
                )
            # alternate PSUM->SBUF drain between DVE and ACT so the two copy
            # pipes overlap the next bank's matmuls
            if nt % 2 == 0:
                nc.vector.tensor_copy(
                    out=y_sb[:, nt * 512 : (nt + 1) * 512], in_=y_banks[nt]
                )
            else:
                nc.scalar.activation(
                    out=y_sb[:, nt * 512 : (nt + 1) * 512], in_=y_banks[nt],
                    func=Copy,
                )
            nc.scalar.dma_start(
                out=out_d[nt],
                in_=y_sb[:, nt * 512 : (nt + 1) * 512],
            )

    nc.compile()
    return nc


_CACHE = {}


def _get_nc(cache_len):
    key = tuple(int(v) for v in cache_len)
    if key not in _CACHE:
        _CACHE[key] = _build(cache_len)
    return _CACHE[key]


def _prep_shards(x, wq, wk, wv, wo, cache_k, cache_v, cache_len):
    import concourse.mybir as mybir

    bf16 = mybir.dt.np(mybir.dt.bfloat16)

    cache_len = np.asarray(cache_len, dtype=np.int32)
    # sort sequences by descending live length: big sequences stream first,
    # small ones land in the drain window; host unpermutes the output rows
    perm = np.argsort(-cache_len, kind="stable")
    cache_len = cache_len[perm]
    x = np.ascontiguousarray(
        np.asarray(x, dtype=np.float32).reshape(B, QL, DIM)[perm].reshape(B * QL, DIM)
    )
    cache_k = cache_k[perm]
    cache_v = cache_v[perm]
    L0s = [int(v) for v in cache_len]
    nJs = [(L + 127) // 128 for L in L0s]

    pos = (cache_len[:, None].astype(np.int64) + np.arange(QL)[None, :]).reshape(-1)
    inv_freq = 1.0 / (THETA ** (np.arange(D // 2, dtype=np.float64) / (D // 2)))
    ang = pos[:, None] * inv_freq[None, :]
    cosb = np.cos(ang).astype(np.float32)
    sinb = np.sin(ang).astype(np.float32)

    # per-b causal masks for the in-stream new keys: new key j sits at probs
    # row RR+j of its chunk; cols are that chunk's 16 (h,i) cols. 1.0 rows
    # elsewhere (gap rows were already -1e30 in the scores).
    nmask = np.ones((128, B, 32), dtype=np.float32)
    for b in range(B):
        rr, _, _, _ = _newkey_layout(L0s[b])
        for j in range(QL):
            p, blk = rr + j, 0
            if p >= 128:
                p, blk = p - 128, 1
            for c in range(16):
                if c % QL < j:
                    nmask[p, b, blk * 16 + c] = 0.0
    nmask = nmask.astype(bf16)

    # K^T per kv-head: [KVH, B, D, KV] in fp8e3 (x2 scale, clipped to the
    # e3m4 range; the device folds the 1/2 into the exp scale). V swizzled
    # bf16 so DMA runs stay long: v_all[c, b, p, s, d] = V[c, b, s*128+p, d]
    f8np = mybir.dt.np(mybir.dt.float8e3)
    kT_all = np.clip(
        np.ascontiguousarray(np.transpose(cache_k, (2, 0, 3, 1))) * 2.0,
        -15.5, 15.5,
    ).astype(f8np)
    v_all = np.clip(
        np.ascontiguousarray(
            np.transpose(
                cache_v.reshape(B, NJMAX, 128, KVH, D), (3, 0, 2, 1, 4)
            )
        )
        * 2.0,
        -15.5, 15.5,
    ).astype(f8np)  # [KVH, B, 128, NJMAX, D], fp8 x2 like the K stream
    k_bases = []
    v_bases = []
    koff = 0
    voff = 0
    for L, nJ in zip(L0s, nJs):
        Lp = (L + 15) // 16 * 16
        k_bases.append(koff)
        koff += 128 * Lp
        v_bases.append(voff)
        voff += 128 * nJ * 128
    k_total = max(koff, 16)
    v_total = max(voff, 16)

    def pack_k8(c):
        buf = np.zeros(k_total, dtype=f8np)
        for b in range(B):
            L = L0s[b]
            if L == 0:
                continue
            Lp = (L + 15) // 16 * 16
            block = buf[k_bases[b] : k_bases[b] + 128 * Lp].reshape(128, Lp)
            block[:, :L] = kT_all[c, b, :, :L]
        return buf

    def pack_v(c):
        buf = np.zeros(v_total, dtype=f8np)
        for b in range(B):
            nJ = nJs[b]
            if nJ == 0:
                continue
            block = buf[v_bases[b] : v_bases[b] + 128 * nJ * 128].reshape(
                128, nJ * 128
            )
            block[:] = v_all[c, b, :, :nJ, :].reshape(128, nJ * D)
        return buf

    xT_host = np.ascontiguousarray(
        x.T.reshape(32, 128, 64).transpose(1, 0, 2)
    ).astype(bf16)

    in_maps = []
    for c in range(N_CORES):
        wk_c = wk[:, c * 128 : (c + 1) * 128].reshape(32, 128, 128)
        wv_c = wv[:, c * 128 : (c + 1) * 128].reshape(32, 128, 128)
        in_maps.append(
            {
                "xT": xT_host,
                "wq": np.ascontiguousarray(wq[:, c * 512 : (c + 1) * 512]).astype(
                    bf16
                ),
                "wk": np.ascontiguousarray(np.transpose(wk_c, (1, 0, 2))).astype(bf16),
                "wv": np.ascontiguousarray(np.transpose(wv_c, (1, 0, 2))).astype(bf16),
                "wo": np.ascontiguousarray(wo[c * 512 : (c + 1) * 512, :]).astype(
                    bf16
                ),
                "k8": pack_k8(c),
                "v": pack_v(c),
                "cosb": cosb,
                "sinb": sinb,
                "nmask": nmask,
                "ident": np.eye(64, dtype=np.float32),
            }
        )
    return in_maps, cache_len, perm


def _run(inputs, trace=False, trace_kwargs=None):
    _install_ntff_hook()
    from concourse.bass_utils import run_bass_kernel_spmd

    in_maps, cache_len, perm = _prep_shards(**inputs)
    nc = _get_nc(cache_len)
    res = run_bass_kernel_spmd(
        nc,
        in_maps,
        core_ids=list(range(N_CORES)),
        trace=trace,
        **(trace_kwargs or {}),
    )
    out_p = np.zeros((B * QL, DIM), dtype=np.float32)
    for i in range(N_CORES):
        ob = np.asarray(res.results[i]["out"], dtype=np.float32)  # [8, 64, 512]
        out_p += np.transpose(ob, (1, 0, 2)).reshape(B * QL, DIM)
    out_p *= 0.5  # V stream (and v_new) carry a x2 scale
    out = np.zeros_like(out_p)
    out.reshape(B, QL, DIM)[perm] = out_p.reshape(B, QL, DIM)
    return out, res


def kernel(**inputs):
    out, _ = _run(inputs, trace=False)
    return out


def kernel_profiled(**inputs):
    out, res = _run(inputs, trace=True)
    return out, res

